# revision 7
# baseline (speedup 1.0000x reference)
"""Trainium2 Bass kernel for nn_Classifier (attribute-sharded MLP heads).

Reference computation (B=64, C=1280, H=W=7, A=40):
    p   = h_swish(mean(x, axis=(2,3)))            # [B, C]
    h   = h_swish(einsum("bc,acd->bad", p, W1) + b1)
    out = sigmoid(einsum("bac,ac->ba", h, W2) + b2)  # [B, A]

Sharding: 8 cores, each owns A/8 = 5 attribute heads (W1/b1/W2/b2 shards);
x is replicated (pre-transposed on host to [C, B*49] so pooling lands in
the matmul-ready [c, b] orientation with zero on-chip transposes).

All large operands are cast to bf16 on host (measured absmax output error
~3e-5 vs fp32 reference; logits are tiny so sigmoid compresses further).
PSUM accumulation stays fp32.
"""

import sys

for _p in ("/opt/trn_rl_repo",):
    if _p not in sys.path:
        sys.path.insert(0, _p)

from contextlib import ExitStack

import numpy as np
import ml_dtypes

import concourse.bass as bass
import concourse.tile as tile
from concourse import bacc, mybir

# Problem constants (hardcoded per contract)
B = 64          # batch
C = 1280        # channels / features
S = 49          # spatial H*W
A = 40          # total attribute heads
NCORES = 8
AH = A // NCORES  # heads per core = 5
P = 128
KC = C // P       # 10 contraction chunks
NS = [(0, 512), (512, 512), (1024, 256)]  # n-chunk (offset, size) of C=1280

BF = mybir.dt.bfloat16
F32 = mybir.dt.float32
AF = mybir.ActivationFunctionType
ALU = mybir.AluOpType

_NC_CACHE = None


def build_nc():
    """Build the per-core Bass program (same program on all 8 cores)."""
    nc = bacc.Bacc("TRN2", target_bir_lowering=False, name="attr_mlp")

    xT = nc.dram_tensor("xT", [C, B * S], BF, kind="ExternalInput")
    w1 = nc.dram_tensor("w1", [AH, C, C], BF, kind="ExternalInput")
    b1 = nc.dram_tensor("b1", [AH * C], BF, kind="ExternalInput")
    w2b = nc.dram_tensor("w2b", [B, AH * C], BF, kind="ExternalInput")
    b2b = nc.dram_tensor("b2b", [B, AH], F32, kind="ExternalInput")
    out = nc.dram_tensor("out", [B, AH], F32, kind="ExternalOutput")

    with tile.TileContext(nc) as tc, ExitStack() as ctx:
        const = ctx.enter_context(tc.tile_pool(name="const", bufs=1))
        xp = ctx.enter_context(tc.tile_pool(name="xp", bufs=3))
        wp = ctx.enter_context(tc.tile_pool(name="wp", bufs=6))
        sp = ctx.enter_context(tc.tile_pool(name="sp", bufs=3))
        pp = ctx.enter_context(tc.tile_pool(name="pp", bufs=2, space="PSUM"))

        # --- constants ---
        ones = const.tile([1, B], BF)
        nc.vector.memset(ones, 1.0)
        half = const.tile([P, 1], F32)  # bias=0.5 for the h_swish Relu
        nc.vector.memset(half, 0.5)
        b1_sb = const.tile([1, AH * C], BF)
        nc.sync.dma_start(b1_sb, b1[None, :])
        w2_sb = const.tile([B, AH * C], BF)
        nc.sync.dma_start(w2_sb, w2b[:, :])
        b2_sb = const.tile([B, AH], F32)
        nc.sync.dma_start(b2_sb, b2b[:, :])

        # pT[ci, k, b] = h_swish(mean(x))[b, 128*k + ci]  (matmul stationary)
        pT = const.tile([P, KC, B], BF)
        # r[b, a*3+ni] = partial dot of head a, n-chunk ni
        r = const.tile([B, AH * 3], F32)

        # --- stage 1: pooling + h_swish -> pT ---
        for k in range(KC):
            xt = xp.tile([P, B * S], BF, tag="xt")
            nc.sync.dma_start(xt, xT[k * P:(k + 1) * P, :])
            sums = sp.tile([P, B], F32, tag="sums")
            nc.vector.reduce_sum(
                sums, xt.rearrange("p (b s) -> p b s", s=S),
                axis=mybir.AxisListType.X,
            )
            # h_swish(p) = p * clip(p/6 + 1/2, 0, 1), p = sums/49
            t1 = sp.tile([P, B], F32, tag="t1")
            nc.scalar.activation(t1, sums, AF.Relu, bias=half, scale=1.0 / (6.0 * 49.0))
            t2 = sp.tile([P, B], F32, tag="t2")
            nc.vector.tensor_scalar(t2, t1, 1.0, 1.0 / 49.0, ALU.min, ALU.mult)
            nc.vector.tensor_tensor(pT[:, k, :], sums, t2, ALU.mult)

        # --- stage 2: per-head GEMM + h_swish + dot(W2) ---
        for a in range(AH):
            ps = pp.tile([B, 3, 512], F32, tag="ps")  # 3 PSUM banks
            for k in range(KC):
                w1kt = wp.tile([P, C], BF, tag="w1kt")
                nc.sync.dma_start(w1kt, w1[a, k * P:(k + 1) * P, :])
                for ni, (n0, nn) in enumerate(NS):
                    nc.tensor.matmul(
                        ps[:, ni, :nn], pT[:, k, :], w1kt[:, n0:n0 + nn],
                        start=(k == 0), stop=False,
                    )
            for ni, (n0, nn) in enumerate(NS):
                # + b1 (outer product ones x b1_chunk), closes the accum group
                nc.tensor.matmul(
                    ps[:, ni, :nn], ones, b1_sb[:, a * C + n0:a * C + n0 + nn],
                    start=False, stop=True,
                )
                # h = z * clip(z/6 + 1/2, 0, 1); r = sum(h * w2)
                t1h = sp.tile([B, 512], F32, tag="t1h")
                nc.scalar.activation(
                    t1h[:, :nn], ps[:, ni, :nn], AF.Relu, bias=half[:B], scale=1.0 / 6.0
                )
                t2h = sp.tile([B, 512], BF, tag="t2h")
                nc.vector.tensor_scalar_min(t2h[:, :nn], t1h[:, :nn], 1.0)
                t2w = sp.tile([B, 512], BF, tag="t2w")
                nc.vector.tensor_tensor(
                    t2w[:, :nn], t2h[:, :nn],
                    w2_sb[:, a * C + n0:a * C + n0 + nn], ALU.mult,
                )
                scr = sp.tile([B, 512], F32, tag="scr")
                nc.vector.tensor_tensor(
                    scr[:, :nn], ps[:, ni, :nn], t2w[:, :nn], ALU.mult
                )
                nc.vector.reduce_sum(
                    r[:, a * 3 + ni:a * 3 + ni + 1], scr[:, :nn],
                    axis=mybir.AxisListType.X,
                )

        # --- finale: sum partials + b2, sigmoid ---
        rs = const.tile([B, AH], F32)
        nc.vector.reduce_sum(
            rs, r.rearrange("b (a n) -> b a n", n=3), axis=mybir.AxisListType.X
        )
        logits = const.tile([B, AH], F32)
        nc.vector.tensor_tensor(logits, rs, b2_sb, ALU.add)
        osb = const.tile([B, AH], F32)
        nc.scalar.activation(osb, logits, AF.Sigmoid)
        nc.sync.dma_start(out[:, :], osb)

    nc.compile()
    return nc


def get_nc():
    global _NC_CACHE
    if _NC_CACHE is None:
        _NC_CACHE = build_nc()
    return _NC_CACHE


def make_in_maps(x, W1, b1, W2, b2):
    bf = ml_dtypes.bfloat16
    x = np.asarray(x, dtype=np.float32)
    W1 = np.asarray(W1, dtype=np.float32)
    b1 = np.asarray(b1, dtype=np.float32)
    W2 = np.asarray(W2, dtype=np.float32)
    b2 = np.asarray(b2, dtype=np.float32)

    # [B, C, H, W] -> [C, B*S], replicated to all cores
    xT = np.ascontiguousarray(
        x.reshape(B, C, S).transpose(1, 0, 2)
    ).reshape(C, B * S).astype(bf)

    in_maps = []
    for core in range(NCORES):
        a0 = core * AH
        w2s = W2[a0:a0 + AH].reshape(1, AH * C).astype(bf)
        in_maps.append({
            "xT": xT,
            "w1": np.ascontiguousarray(W1[a0:a0 + AH]).astype(bf),
            "b1": np.ascontiguousarray(b1[a0:a0 + AH]).reshape(AH * C).astype(bf),
            "w2b": np.ascontiguousarray(np.broadcast_to(w2s, (B, AH * C))),
            "b2b": np.ascontiguousarray(
                np.broadcast_to(b2[a0:a0 + AH].reshape(1, AH), (B, AH))
            ).astype(np.float32),
        })
    return in_maps


def kernel(x, W1, b1, W2, b2, _trace=False, _tmpdir=None):
    from concourse.bass_utils import run_bass_kernel_spmd

    nc = get_nc()
    in_maps = make_in_maps(x, W1, b1, W2, b2)
    res = run_bass_kernel_spmd(
        nc, in_maps, core_ids=list(range(NCORES)),
        trace=_trace, tmpdir=_tmpdir,
    )
    outs = [np.asarray(res.results[c]["out"], dtype=np.float32)
            for c in range(NCORES)]
    full = np.concatenate(outs, axis=1)  # [B, A]
    if _trace:
        return full, res
    return full


# revision 8
# speedup vs baseline: 26.1303x; 26.1303x over previous
"""Trainium2 Bass kernel for nn_Classifier (attribute-sharded MLP heads).

Reference computation (B=64, C=1280, H=W=7, A=40):
    p   = h_swish(mean(x, axis=(2,3)))            # [B, C]
    h   = h_swish(einsum("bc,acd->bad", p, W1) + b1)
    out = sigmoid(einsum("bac,ac->ba", h, W2) + b2)  # [B, A]

Sharding: 8 cores, each owns A/8 = 5 attribute heads (W1/b1/W2/b2 shards);
x is replicated (pre-transposed on host to [C, B*49] so pooling lands in
the matmul-ready [c, b] orientation with zero on-chip transposes).

All large operands are cast to bf16 on host (measured absmax output error
~3e-5 vs fp32 reference; logits are tiny so sigmoid compresses further).
PSUM accumulation stays fp32.
"""

import sys

for _p in ("/opt/trn_rl_repo",):
    if _p not in sys.path:
        sys.path.insert(0, _p)

from contextlib import ExitStack

import numpy as np
import ml_dtypes

import concourse.bass as bass
import concourse.tile as tile
from concourse import bacc, mybir

# Problem constants (hardcoded per contract)
B = 64          # batch
C = 1280        # channels / features
S = 49          # spatial H*W
A = 40          # total attribute heads
NCORES = 8
AH = A // NCORES  # heads per core = 5
P = 128
KC = C // P       # 10 contraction chunks
NS = [(0, 512), (512, 512), (1024, 256)]  # n-chunk (offset, size) of C=1280

BF = mybir.dt.bfloat16
F32 = mybir.dt.float32
AF = mybir.ActivationFunctionType
ALU = mybir.AluOpType

_NC_CACHE = {}


def build_nc(reps=1):
    """Build the per-core Bass program (same program on all 8 cores).

    reps>1 unrolls the whole computation back-to-back (same inputs,
    same output) — used only for steady-state throughput benchmarking.
    """
    nc = bacc.Bacc("TRN2", target_bir_lowering=False, name="attr_mlp")

    xT = nc.dram_tensor("xT", [C, B * S], BF, kind="ExternalInput")
    w1 = nc.dram_tensor("w1", [AH, C, C], BF, kind="ExternalInput")
    b1 = nc.dram_tensor("b1", [AH * C], BF, kind="ExternalInput")
    w2b = nc.dram_tensor("w2b", [B, AH * C], BF, kind="ExternalInput")
    b2b = nc.dram_tensor("b2b", [B, AH], F32, kind="ExternalInput")
    out = nc.dram_tensor("out", [B, AH], F32, kind="ExternalOutput")

    with tile.TileContext(nc) as tc, ExitStack() as ctx:
        const = ctx.enter_context(tc.tile_pool(name="const", bufs=1))
        st = ctx.enter_context(tc.tile_pool(name="st", bufs=2))
        xp = ctx.enter_context(tc.tile_pool(name="xp", bufs=3))
        wp = ctx.enter_context(tc.tile_pool(name="wp", bufs=6))
        sp = ctx.enter_context(tc.tile_pool(name="sp", bufs=3))
        pp = ctx.enter_context(tc.tile_pool(name="pp", bufs=2, space="PSUM"))

        # --- constants (loaded once) ---
        ones = const.tile([1, B], BF)
        nc.vector.memset(ones, 1.0)
        half = const.tile([P, 1], F32)  # bias=0.5 for the h_swish Relu
        nc.vector.memset(half, 0.5)
        b1_sb = const.tile([1, AH * C], BF)
        nc.sync.dma_start(b1_sb, b1[None, :])
        w2_sb = const.tile([B, AH * C], BF)
        nc.sync.dma_start(w2_sb, w2b[:, :])
        b2_sb = const.tile([B, AH], F32)
        nc.sync.dma_start(b2_sb, b2b[:, :])

        for _rep in range(reps):
            # pT[ci, k, b] = h_swish(mean(x))[b, 128*k + ci]  (matmul lhsT)
            pT = st.tile([P, KC, B], BF, tag="pT")
            # r[b, a*3+ni] = partial dot of head a, n-chunk ni
            r = st.tile([B, AH * 3], F32, tag="r")

            # --- stage 1: pooling + h_swish -> pT ---
            for k in range(KC):
                xt = xp.tile([P, B * S], BF, tag="xt")
                nc.sync.dma_start(xt, xT[k * P:(k + 1) * P, :])
                sums = sp.tile([P, B], F32, tag="sums")
                nc.vector.reduce_sum(
                    sums, xt.rearrange("p (b s) -> p b s", s=S),
                    axis=mybir.AxisListType.X,
                )
                # h_swish(p) = p * clip(p/6 + 1/2, 0, 1), p = sums/49
                t1 = sp.tile([P, B], F32, tag="t1")
                nc.scalar.activation(
                    t1, sums, AF.Relu, bias=half, scale=1.0 / (6.0 * 49.0)
                )
                t2 = sp.tile([P, B], F32, tag="t2")
                nc.vector.tensor_scalar(t2, t1, 1.0, 1.0 / 49.0, ALU.min, ALU.mult)
                nc.vector.tensor_tensor(pT[:, k, :], sums, t2, ALU.mult)

            # --- stage 2: per-head GEMM + h_swish + dot(W2) ---
            for a in range(AH):
                ps = pp.tile([B, 3, 512], F32, tag="ps")  # 3 PSUM banks
                for k in range(KC):
                    w1kt = wp.tile([P, C], BF, tag="w1kt")
                    nc.sync.dma_start(w1kt, w1[a, k * P:(k + 1) * P, :])
                    for ni, (n0, nn) in enumerate(NS):
                        nc.tensor.matmul(
                            ps[:, ni, :nn], pT[:, k, :], w1kt[:, n0:n0 + nn],
                            start=(k == 0), stop=False,
                        )
                for ni, (n0, nn) in enumerate(NS):
                    # + b1 (outer product ones x b1_chunk); closes the group
                    nc.tensor.matmul(
                        ps[:, ni, :nn], ones,
                        b1_sb[:, a * C + n0:a * C + n0 + nn],
                        start=False, stop=True,
                    )
                    # h = z * clip(z/6 + 1/2, 0, 1); r = sum(h * w2)
                    t1h = sp.tile([B, 512], F32, tag="t1h")
                    nc.scalar.activation(
                        t1h[:, :nn], ps[:, ni, :nn], AF.Relu,
                        bias=half[:B], scale=1.0 / 6.0,
                    )
                    t2h = sp.tile([B, 512], BF, tag="t2h")
                    nc.vector.tensor_scalar_min(t2h[:, :nn], t1h[:, :nn], 1.0)
                    t2w = sp.tile([B, 512], BF, tag="t2w")
                    nc.vector.tensor_tensor(
                        t2w[:, :nn], t2h[:, :nn],
                        w2_sb[:, a * C + n0:a * C + n0 + nn], ALU.mult,
                    )
                    scr = sp.tile([B, 512], F32, tag="scr")
                    nc.vector.tensor_tensor(
                        scr[:, :nn], ps[:, ni, :nn], t2w[:, :nn], ALU.mult
                    )
                    nc.vector.reduce_sum(
                        r[:, a * 3 + ni:a * 3 + ni + 1], scr[:, :nn],
                        axis=mybir.AxisListType.X,
                    )

            # --- finale: sum partials + b2, sigmoid ---
            rs = st.tile([B, AH], F32, tag="rs")
            nc.vector.reduce_sum(
                rs, r.rearrange("b (a n) -> b a n", n=3),
                axis=mybir.AxisListType.X,
            )
            logits = st.tile([B, AH], F32, tag="logits")
            nc.vector.tensor_tensor(logits, rs, b2_sb, ALU.add)
            osb = st.tile([B, AH], F32, tag="osb")
            nc.scalar.activation(osb, logits, AF.Sigmoid)
            nc.sync.dma_start(out[:, :], osb)

    nc.compile()
    return nc


def get_nc(reps=1):
    if reps not in _NC_CACHE:
        _NC_CACHE[reps] = build_nc(reps)
    return _NC_CACHE[reps]


def make_in_maps(x, W1, b1, W2, b2):
    bf = ml_dtypes.bfloat16
    x = np.asarray(x, dtype=np.float32)
    W1 = np.asarray(W1, dtype=np.float32)
    b1 = np.asarray(b1, dtype=np.float32)
    W2 = np.asarray(W2, dtype=np.float32)
    b2 = np.asarray(b2, dtype=np.float32)

    # [B, C, H, W] -> [C, B*S], replicated to all cores
    xT = np.ascontiguousarray(
        x.reshape(B, C, S).transpose(1, 0, 2)
    ).reshape(C, B * S).astype(bf)

    in_maps = []
    for core in range(NCORES):
        a0 = core * AH
        w2s = W2[a0:a0 + AH].reshape(1, AH * C).astype(bf)
        in_maps.append({
            "xT": xT,
            "w1": np.ascontiguousarray(W1[a0:a0 + AH]).astype(bf),
            "b1": np.ascontiguousarray(b1[a0:a0 + AH]).reshape(AH * C).astype(bf),
            "w2b": np.ascontiguousarray(np.broadcast_to(w2s, (B, AH * C))),
            "b2b": np.ascontiguousarray(
                np.broadcast_to(b2[a0:a0 + AH].reshape(1, AH), (B, AH))
            ).astype(np.float32),
        })
    return in_maps


def kernel(x, W1, b1, W2, b2, _trace=False, _tmpdir=None):
    from concourse.bass_utils import run_bass_kernel_spmd

    nc = get_nc()
    in_maps = make_in_maps(x, W1, b1, W2, b2)
    res = run_bass_kernel_spmd(
        nc, in_maps, core_ids=list(range(NCORES)),
        trace=_trace, tmpdir=_tmpdir,
    )
    outs = [np.asarray(res.results[c]["out"], dtype=np.float32)
            for c in range(NCORES)]
    full = np.concatenate(outs, axis=1)  # [B, A]
    if _trace:
        return full, res
    return full
